# revision 1
# baseline (speedup 1.0000x reference)
"""Trainium2 Bass kernel for causal multi-head attention block.

Reference computation (B=4, S=2048, D=1024, H=16, HD=64, fp32):
    qkv = x @ Wqkv + bqkv; split q,k,v; per-head scaled scores;
    causal mask filled with -0.0001 (leaky, NOT -inf); softmax over all
    2048 keys; out = P @ V; out = out @ Wo + bo.

Sharding: 8 cores, core = (batch b = i//2, parity p = i%2). Each core
computes 1024 queries of its batch: query tiles t = 2j+p (j=0..3) of
256 queries. Causal block structure (512-key blocks per 256-query tile)
is then identical on every core: qtile j needs key blocks 0..j -> one
SPMD program, zero cross-core communication.

The leaky mask is handled exactly:
  - computed blocks: E' = exp(S)*M + (1-M)*w  with w = exp(-1e-4)
  - skipped key blocks (all masked): contribute w*SufV[j] to the
    numerator (suffix sums of V at 512-block granularity) and
    w*n_skip to the denominator Z.
Z is produced inside the PV matmul via a 65th all-ones V column.
Matmuls run as float32r (full-rate fp32 streaming); Q/K are stored
bf16 so the dh=64 score matmuls stream at full fetch rate.
"""

import math
from contextlib import ExitStack

import numpy as np

import concourse.bass as bass
import concourse.mybir as mybir
import concourse.tile as tile
from concourse import bacc

F32 = mybir.dt.float32
F32R = mybir.dt.float32r
BF16 = mybir.dt.bfloat16
AF = mybir.ActivationFunctionType
ALU = mybir.AluOpType
AX = mybir.AxisListType

B, S, D, H, HD = 4, 2048, 1024, 16, 64
QL, QT, KB, NJ = 1024, 256, 512, 4    # queries/core, qtile, key block, n qtiles
NCH = D // 128                         # contraction chunks
PAIRS = H // 2
W_MASK = math.exp(-1e-4)


def _r(ap):
    return ap


def build_program():
    nc = bacc.Bacc(
        "TRN2",
        target_bir_lowering=False,
        debug=False,
        num_devices=8,
    )
    xT = nc.declare_dram_parameter("xT", [D, S], F32R, isOutput=False)
    xqT = nc.declare_dram_parameter("xqT", [D, QL], F32R, isOutput=False)
    wqkv = nc.declare_dram_parameter("wqkv", [D, 3 * D], F32R, isOutput=False)
    wo = nc.declare_dram_parameter("wo", [D, D], F32R, isOutput=False)
    b2h = nc.declare_dram_parameter("b2h", [128, 24], F32, isOutput=False)
    brow = nc.declare_dram_parameter("brow", [1, 3 * D], F32R, isOutput=False)
    bv512 = nc.declare_dram_parameter("bv512", [128, 8], F32, isOutput=False)
    bocol = nc.declare_dram_parameter("bocol", [128, 8], F32, isOutput=False)
    mmul = nc.declare_dram_parameter("mmul", [128, 4 * QT], F32R, isOutput=False)
    madd = nc.declare_dram_parameter("madd", [128, 4 * QT], F32R, isOutput=False)
    onesd = nc.declare_dram_parameter("onesd", [128, 128], F32R, isOutput=False)
    outT = nc.declare_dram_parameter("outT", [D, QL], F32, isOutput=True)

    with tile.TileContext(nc) as tc, ExitStack() as ctx, \
         nc.allow_low_precision(reason="float32r matmul inputs are fp32 bits"):
        consts = ctx.enter_context(tc.tile_pool(name="consts", bufs=1))
        b2h_sb = consts.tile([128, 24], F32)
        nc.sync.dma_start(out=b2h_sb, in_=b2h[:])
        brow_sb = consts.tile([1, D], F32R)
        nc.sync.dma_start(out=brow_sb, in_=brow[0:1, 2 * D:3 * D])
        bv512_sb = consts.tile([128, 8], F32)
        nc.sync.dma_start(out=bv512_sb, in_=bv512[:])
        bocol_sb = consts.tile([128, 8], F32)
        nc.sync.dma_start(out=bocol_sb, in_=bocol[:])
        mmul_sb = consts.tile([128, 4 * QT], F32R)
        nc.sync.dma_start(out=mmul_sb, in_=mmul[:])
        madd_sb = consts.tile([128, 4 * QT], F32R)
        nc.sync.dma_start(out=madd_sb, in_=madd[:])
        ones_sb = consts.tile([1, 128], F32R)
        nc.sync.dma_start(out=ones_sb, in_=onesd[0:1, :])

        with ExitStack() as ctx2:
            xt_pool = ctx2.enter_context(tc.tile_pool(name="xt", bufs=1))
            xT_sb = xt_pool.tile([128, NCH, S], F32R)
            for c in range(NCH):
                nc.sync.dma_start(out=xT_sb[:, c, :], in_=xT[128 * c:128 * (c + 1), :])

            # per-512-block column sums of xT (for V block-sum corrections)
            xsum_sb = consts.tile([128, NCH, 4], F32R)
            for c in range(NCH):
                nc.vector.tensor_reduce(
                    out=xsum_sb[:, c, :],
                    in_=xT_sb[:, c, :].rearrange("p (b t) -> p b t", b=4),
                    axis=AX.X, op=ALU.add,
                )

            # ---------- Q projection, all head pairs up front ----------
            qt_pool = ctx2.enter_context(tc.tile_pool(name="qt", bufs=1))
            QT_all = qt_pool.tile([128, PAIRS, QL], BF16)
            with tc.tile_pool(name="xq", bufs=1) as xq_pool, \
                 tc.tile_pool(name="wq", bufs=2) as wq_pool, \
                 tc.tile_pool(name="qps", bufs=2, space="PSUM") as qps_pool:
                xqT_sb = xq_pool.tile([128, NCH, QL], F32R)
                for c in range(NCH):
                    nc.sync.dma_start(out=xqT_sb[:, c, :], in_=xqT[128 * c:128 * (c + 1), :])
                for pr in range(PAIRS):
                    wq_sb = wq_pool.tile([128, NCH, 128], F32R)
                    nc.sync.dma_start(
                        out=wq_sb,
                        in_=wqkv[:, 128 * pr:128 * (pr + 1)].rearrange("(c p) m -> p c m", p=128),
                    )
                    for g2 in range(2):
                        ps = qps_pool.tile([128, 512], F32)
                        for c in range(NCH):
                            nc.tensor.matmul(
                                out=ps, lhsT=_r(wq_sb[:, c, :]),
                                rhs=_r(xqT_sb[:, c, 512 * g2:512 * (g2 + 1)]),
                                start=(c == 0), stop=(c == NCH - 1),
                            )
                        # QT = (x@Wq)*0.125 + bq/8  (bias columns pre-divided on host)
                        nc.vector.tensor_scalar(
                            out=QT_all[:, pr, 512 * g2:512 * (g2 + 1)], in0=ps,
                            scalar1=0.125, scalar2=b2h_sb[:, pr:pr + 1],
                            op0=ALU.mult, op1=ALU.add,
                        )

            # ---------- main loop: 4 groups of 4 heads ----------
            vpool = ctx2.enter_context(tc.tile_pool(name="vsb", bufs=2))
            kt_pool = ctx2.enter_context(tc.tile_pool(name="kt", bufs=2))
            odram = ctx2.enter_context(tc.tile_pool(name="odram", bufs=1, space="DRAM"))
            O_dr = odram.tile([D, QL], F32R)       # [h*d, q] transposed head outputs

            with tc.tile_pool(name="wv", bufs=2) as wv_pool, \
                 tc.tile_pool(name="wk", bufs=2) as wk_pool, \
                 tc.tile_pool(name="pps", bufs=2, space="PSUM") as pps_pool, \
                 tc.tile_pool(name="sps", bufs=2, space="PSUM") as sps_pool, \
                 tc.tile_pool(name="ops", bufs=2, space="PSUM") as ops_pool, \
                 tc.tile_pool(name="esb", bufs=4) as e_pool, \
                 tc.tile_pool(name="bs", bufs=2) as bs_pool, \
                 tc.tile_pool(name="osb", bufs=4) as osb_pool, \
                 tc.tile_pool(name="misc", bufs=4) as misc_pool:

                for g in range(4):
                    # V projection for this group's 4 heads (token-major, 65th ones col)
                    wv_sb = wv_pool.tile([128, NCH, 256], F32R)
                    nc.sync.dma_start(
                        out=wv_sb,
                        in_=wqkv[:, 2 * D + 256 * g:2 * D + 256 * (g + 1)].rearrange("(c p) m -> p c m", p=128),
                    )
                    V_sb = vpool.tile([128, 16, 4, 65], F32R)
                    nc.sync.dma_start(
                        out=V_sb[:, :, :, 64],
                        in_=onesd[:, 0:64].rearrange("p (t g) -> p t g", t=16),
                    )
                    for t in range(16):
                        ps = pps_pool.tile([128, 256], F32, tag="pps")
                        for c in range(NCH):
                            nc.tensor.matmul(
                                out=ps, lhsT=_r(xT_sb[:, c, 128 * t:128 * (t + 1)]),
                                rhs=_r(wv_sb[:, c, :]),
                                start=(c == 0), stop=False,
                            )
                        nc.tensor.matmul(
                            out=ps, lhsT=_r(ones_sb),
                            rhs=_r(brow_sb[:, 256 * g:256 * (g + 1)]),
                            start=False, stop=True,
                        )
                        nc.vector.tensor_copy(
                            out=V_sb[:, t, :, 0:64],
                            in_=ps.rearrange("p (h d) -> p h d", h=4),
                        )

                    for lp in range(2):
                        pr = 2 * g + lp
                        # W-scaled per-block V column sums -> suffix sums
                        psb = pps_pool.tile([128, 4], F32, tag="pps")
                        for c in range(NCH):
                            nc.tensor.matmul(
                                out=psb, lhsT=_r(wv_sb[:, c, 128 * lp:128 * (lp + 1)]),
                                rhs=_r(xsum_sb[:, c, :]),
                                start=(c == 0), stop=(c == NCH - 1),
                            )
                        bs_sb = bs_pool.tile([128, 4], F32, tag="bs")
                        nc.vector.tensor_scalar(
                            out=bs_sb, in0=psb, scalar1=W_MASK,
                            scalar2=bv512_sb[:, pr:pr + 1], op0=ALU.mult, op1=ALU.add,
                        )
                        suf_sb = bs_pool.tile([128, 4], F32, tag="suf")
                        nc.vector.memset(suf_sb[:, 3:4], 0.0)
                        nc.vector.tensor_copy(out=suf_sb[:, 2:3], in_=bs_sb[:, 3:4])
                        nc.vector.tensor_add(out=suf_sb[:, 1:2], in0=bs_sb[:, 2:3], in1=suf_sb[:, 2:3])
                        nc.vector.tensor_add(out=suf_sb[:, 0:1], in0=bs_sb[:, 1:2], in1=suf_sb[:, 1:2])

                        # K projection for this pair (d-major)
                        wk_sb = wk_pool.tile([128, NCH, 128], F32R)
                        nc.sync.dma_start(
                            out=wk_sb,
                            in_=wqkv[:, D + 128 * pr:D + 128 * (pr + 1)].rearrange("(c p) m -> p c m", p=128),
                        )
                        KT_sb = kt_pool.tile([128, S], BF16)
                        for kg in range(4):
                            ps = pps_pool.tile([128, 512], F32, tag="pps")
                            for c in range(NCH):
                                nc.tensor.matmul(
                                    out=ps, lhsT=_r(wk_sb[:, c, :]),
                                    rhs=_r(xT_sb[:, c, 512 * kg:512 * (kg + 1)]),
                                    start=(c == 0), stop=(c == NCH - 1),
                                )
                            nc.vector.tensor_scalar_add(
                                out=KT_sb[:, 512 * kg:512 * (kg + 1)], in0=ps,
                                scalar1=b2h_sb[:, 8 + pr:9 + pr],
                            )

                        # attention, 2 heads interleaved per qtile to keep PE dense
                        for j in range(NJ):
                            for hl in range(2):
                                ghl = 2 * lp + hl
                                hsl = slice(64 * hl, 64 * (hl + 1))
                                # po cols 0:256 = PV accum + Z row; cols 256:512 = Z broadcast
                                po = ops_pool.tile([65, 512], F32, tag="ops")
                                for kb in range(j + 1):
                                    diag = kb == j
                                    pss = sps_pool.tile([128, 4, 256], F32)
                                    for s2 in range(4):
                                        nc.tensor.matmul(
                                            out=pss[:, s2, :],
                                            lhsT=_r(KT_sb[hsl, 512 * kb + 128 * s2:512 * kb + 128 * (s2 + 1)]),
                                            rhs=_r(QT_all[hsl, pr, 256 * j:256 * (j + 1)]),
                                            start=True, stop=True,
                                        )
                                    e_sb = e_pool.tile([128, 4, 256], F32R)
                                    nc.scalar.activation(out=e_sb, in_=pss, func=AF.Exp)
                                    if diag:
                                        ef = e_sb[:].rearrange("p a b -> p (a b)")
                                        nc.vector.tensor_mul(out=ef, in0=ef, in1=mmul_sb[:])
                                        nc.vector.tensor_add(out=ef, in0=ef, in1=madd_sb[:])
                                    for s2 in range(4):
                                        nc.tensor.matmul(
                                            out=po[:, 0:256],
                                            lhsT=_r(V_sb[:, 4 * kb + s2, ghl, :]),
                                            rhs=_r(e_sb[:, s2, :]),
                                            start=(kb == 0 and s2 == 0),
                                            stop=(kb == j and s2 == 3),
                                            skip_group_check=True,
                                        )
                                # epilogue: Z, broadcast, numerator correction, divide
                                nskip = S - KB * (j + 1)
                                zf = misc_pool.tile([1, 256], F32, tag="zf")
                                nc.vector.tensor_scalar_add(out=zf, in0=po[64:65, 0:256], scalar1=W_MASK * nskip)
                                zi = misc_pool.tile([1, 256], F32, tag="zi")
                                nc.vector.reciprocal_approx_fast(out=zi, in_=zf)
                                zr = misc_pool.tile([1, 256], F32R, tag="zr")
                                nc.vector.tensor_copy(out=zr, in_=zi)
                                nc.tensor.matmul(out=po[0:64, 256:512], lhsT=_r(ones_sb[:, 0:64]), rhs=_r(zr), start=True, stop=True)
                                nm = misc_pool.tile([64, 256], F32, tag="nm")
                                nc.vector.tensor_scalar_add(
                                    out=nm, in0=po[0:64, 0:256], scalar1=suf_sb[hsl, j:j + 1],
                                )
                                ot = osb_pool.tile([64, 256], F32R, tag="ot")
                                nc.vector.tensor_mul(out=ot, in0=nm, in1=po[0:64, 256:512])
                                nc.sync.dma_start(
                                    out=O_dr[128 * pr + 64 * hl:128 * pr + 64 * (hl + 1), 256 * j:256 * (j + 1)],
                                    in_=ot,
                                )

        # ---------- output projection ----------
        with tc.tile_pool(name="wosb", bufs=1) as wo_pool, \
             tc.tile_pool(name="ochunk", bufs=2) as oc_pool, \
             tc.tile_pool(name="fps", bufs=2, space="PSUM") as fps_pool, \
             tc.tile_pool(name="fout", bufs=3) as fo_pool:
            wo_sb = wo_pool.tile([128, NCH, 8, 128], F32R)
            nc.sync.dma_start(
                out=wo_sb,
                in_=wo[:].rearrange("(c p) (t m) -> p c t m", p=128, m=128),
            )
            for j in range(NJ):
                oj = oc_pool.tile([128, NCH, 256], F32R)
                nc.sync.dma_start(
                    out=oj,
                    in_=O_dr[:, 256 * j:256 * (j + 1)].rearrange("(c p) q -> p c q", p=128),
                )
                for dt_ in range(8):
                    ps = fps_pool.tile([128, 256], F32)
                    for c in range(NCH):
                        nc.tensor.matmul(
                            out=ps, lhsT=_r(wo_sb[:, c, dt_, :]), rhs=_r(oj[:, c, :]),
                            start=(c == 0), stop=(c == NCH - 1),
                        )
                    fo = fo_pool.tile([128, 256], F32)
                    nc.vector.tensor_scalar_add(out=fo, in0=ps, scalar1=bocol_sb[:, dt_:dt_ + 1])
                    nc.sync.dma_start(
                        out=outT[128 * dt_:128 * (dt_ + 1), 256 * j:256 * (j + 1)],
                        in_=fo,
                    )
    nc.compile()
    return nc


def qrows_for(p):
    return np.concatenate(
        [np.arange(QT * (2 * j + p), QT * (2 * j + p) + QT) for j in range(NJ)]
    )


def host_in_maps(x, Wqkv, bqkv, Wo, bo):
    x = np.ascontiguousarray(np.asarray(x, np.float32))
    Wqkv = np.ascontiguousarray(np.asarray(Wqkv, np.float32))
    bqkv = np.asarray(bqkv, np.float32)
    Wo = np.ascontiguousarray(np.asarray(Wo, np.float32))
    bo = np.asarray(bo, np.float32)

    b2h = np.ascontiguousarray(bqkv.reshape(24, 128).T)
    b2h[:, 0:8] /= 8.0
    brow = bqkv.reshape(1, 3 * D)
    bv512 = np.ascontiguousarray((W_MASK * 512.0 * bqkv[2 * D:].reshape(8, 128)).T)
    bocol = np.ascontiguousarray(bo.reshape(8, 128).T)
    onesd = np.ones((128, 128), np.float32)

    kap = np.arange(128)[:, None]
    r = np.arange(QT)[None, :]
    masks = {}
    for p in range(2):
        mm = np.zeros((128, 4, QT), np.float32)
        for s in range(4):
            mm[:, s, :] = (128 * s + kap <= QT * p + r)
        mm2 = np.ascontiguousarray(mm.reshape(128, 4 * QT))
        masks[p] = (mm2, np.ascontiguousarray((1.0 - mm2) * W_MASK))

    in_maps = []
    for core in range(8):
        b, p = core // 2, core % 2
        mma, mada = masks[p]
        in_maps.append({
            "xT": np.ascontiguousarray(x[b].T),
            "xqT": np.ascontiguousarray(x[b][qrows_for(p)].T),
            "wqkv": Wqkv,
            "wo": Wo,
            "b2h": b2h,
            "brow": brow,
            "bv512": bv512,
            "bocol": bocol,
            "onesd": onesd,
            "mmul": mma,
            "madd": mada,
        })
    return in_maps


_CACHED = {}


def get_program():
    if "nc" not in _CACHED:
        _CACHED["nc"] = build_program()
    return _CACHED["nc"]


def kernel(x, Wqkv, bqkv, Wo, bo):
    from concourse.bass_utils import run_bass_kernel_spmd

    nc = get_program()
    in_maps = host_in_maps(x, Wqkv, bqkv, Wo, bo)
    res = run_bass_kernel_spmd(nc, in_maps, core_ids=list(range(8)))
    out = np.zeros((B, S, D), np.float32)
    for core in range(8):
        b, p = core // 2, core % 2
        out[b, qrows_for(p), :] = res.results[core]["outT"].T
    return out



# revision 8
# speedup vs baseline: 1.1487x; 1.1487x over previous
"""Trainium2 Bass kernel for causal multi-head attention block.

Reference computation (B=4, S=2048, D=1024, H=16, HD=64, fp32):
    qkv = x @ Wqkv + bqkv; split q,k,v; per-head scaled scores;
    causal mask filled with -0.0001 (leaky, NOT -inf); softmax over all
    2048 keys; out = P @ V; out = out @ Wo + bo.

Sharding (head-split tensor parallel): core i = (batch b = i//2,
head half p = i%2). Each core computes ALL 2048 queries of its batch
for heads 8p..8p+7: QKV projections column-sharded by head, attention
device-local, output projection row-sharded (contraction over this
core's 512 head-dims) -> partial outputs. The two partials per batch
are summed at unshard time (host gather). The V bias is absorbed into
the per-core output bias: out_head = P@V0 + bv exactly (softmax rows
sum to 1), so bv contributes bv @ Wo_mine.

Leaky-mask algebra (w = exp(-1e-4)):
  - scores per 512-query tile t against key blocks 0..t; the diagonal
    block is split so the fully-masked upper 256-key piece of the
    first query half is never computed.
  - masked chunks: S' = (S + 8e-4) * M fused on PSUM (one DVE op),
    then exp(0.125*S' - 1e-4) = exp(S/8) unmasked / w masked.
  - skipped key blocks contribute w*Suf[d] to the numerator (suffix
    sums of unbiased V at 256-block granularity) and w*nskip to Z.
Z comes from a 65th all-ones V column in the PV matmul; 1/Z is
broadcast across the 64 head-dims with a rank-1 PE matmul. Score
matmuls for the 2 heads of a pair run concurrently via PE row tiling
(64-partition tiles at rows 0/64). Scores->exp->PV is software-
pipelined (lag 1 chunk) and Q/K/V/wo setup for later pairs is
interleaved into earlier pairs' attention as filler PE work.
"""

import math
from contextlib import ExitStack

import numpy as np

import concourse.bass as bass
import concourse.mybir as mybir
import concourse.tile as tile
from concourse import bacc

F32 = mybir.dt.float32
F32R = mybir.dt.float32r
BF16 = mybir.dt.bfloat16
AF = mybir.ActivationFunctionType
ALU = mybir.AluOpType
AX = mybir.AxisListType

B, S, D, H, HD = 4, 2048, 1024, 16, 64
HPC = 8            # heads per core
NP = 4             # head pairs per core
NCH = D // 128     # contraction chunks
NT = 4             # 512-query tiles
W_MASK = math.exp(-1e-4)


def build_program():
    nc = bacc.Bacc(
        "TRN2",
        target_bir_lowering=False,
        debug=False,
        num_devices=8,
    )
    xT = nc.declare_dram_parameter("xT", [D, S], F32R, isOutput=False)
    wq = nc.declare_dram_parameter("wq", [D, 512], F32R, isOutput=False)
    wk = nc.declare_dram_parameter("wk", [D, 512], F32R, isOutput=False)
    wv = nc.declare_dram_parameter("wv", [D, 512], F32R, isOutput=False)
    wo = nc.declare_dram_parameter("wo", [512, D], F32, isOutput=False)
    bqk = nc.declare_dram_parameter("bqk", [128, 8], F32, isOutput=False)
    bocol = nc.declare_dram_parameter("bocol", [128, 8], F32, isOutput=False)
    mmul = nc.declare_dram_parameter("mmul", [128, 2, 768], BF16, isOutput=False)
    outT = nc.declare_dram_parameter("outT", [D, S], F32, isOutput=True)

    with tile.TileContext(nc) as tc, ExitStack() as ctx, \
         nc.allow_low_precision(reason="float32r matmul inputs are fp32 bits"):
        consts = ctx.enter_context(tc.tile_pool(name="consts", bufs=1))
        bqk_sb = consts.tile([128, 8], F32)
        nc.sync.dma_start(out=bqk_sb, in_=bqk[:])
        bocol_sb = consts.tile([128, 8], F32)
        nc.sync.dma_start(out=bocol_sb, in_=bocol[:])
        mmul_sb = consts.tile([128, 2, 768], BF16)
        nc.sync.dma_start(out=mmul_sb, in_=mmul[:])
        onef = consts.tile([128, 128], F32)
        nc.vector.memset(onef, 1.0)
        ones_bf = consts.tile([1, 64], BF16)
        nc.vector.tensor_copy(out=ones_bf, in_=onef[0:1, 0:64])
        bias_neg = consts.tile([128, 1], F32)
        nc.vector.memset(bias_neg, -1e-4)

        xt_pool = ctx.enter_context(tc.tile_pool(name="xt", bufs=1))
        xT_sb = xt_pool.tile([128, NCH, S], F32R)
        for c in range(NCH):
            nc.sync.dma_start(out=xT_sb[:, c, :], in_=xT[128 * c:128 * (c + 1), :])

        # per-256-block column sums of xT (for V suffix corrections)
        xsum_sb = consts.tile([128, NCH, 8], F32R)
        for c in range(NCH):
            nc.vector.tensor_reduce(
                out=xsum_sb[:, c, :],
                in_=xT_sb[:, c, :].rearrange("p (b t) -> p b t", b=8),
                axis=AX.X, op=ALU.add,
            )

        # persistent attention-side tensors
        big = ctx.enter_context(tc.tile_pool(name="big", bufs=1))
        V_sb = big.tile([128, 16, HPC, 65], F32R)  # [key sub, tok blk, head, d+1]
        O_sb = big.tile([128, NP, S], BF16)        # [2 heads x 64, chunk(=pair), q]
        suf_sb = big.tile([64, NP, 2, 9], F32)     # [d, pair, head, 256-block idx]
        wo_bf = big.tile([128, NP, 8, 128], BF16)
        nc.vector.tensor_copy(
            out=V_sb[:, :, :, 64],
            in_=onef.rearrange("p (a b) -> p a b", a=16)[:, :, 0:8])

        wv_pool = ctx.enter_context(tc.tile_pool(name="wvp", bufs=1))
        wv_sb = wv_pool.tile([128, NCH, 512], F32R)
        nc.sync.dma_start(
            out=wv_sb, in_=wv[:].rearrange("(c p) m -> p c m", p=128))

        qk_ring = ctx.enter_context(tc.tile_pool(name="qkr", bufs=2))
        w_ring = ctx.enter_context(tc.tile_pool(name="wr", bufs=2))
        wof = ctx.enter_context(tc.tile_pool(name="wof", bufs=1))

        qt_tiles = {}
        kt_tiles = {}
        w_tiles = {}

        with tc.tile_pool(name="pps", bufs=2, space="PSUM") as pps, \
             tc.tile_pool(name="sps", bufs=2, space="PSUM") as sps, \
             tc.tile_pool(name="pop", bufs=2, space="PSUM") as pop, \
             tc.tile_pool(name="epool", bufs=3) as epool, \
             tc.tile_pool(name="misc", bufs=2) as misc:

            # ---------------- deferred setup tasks ----------------
            def v_task(g, t):
                # V proj for head group g (4 heads), token block t
                def run():
                    ps = pps.tile([128, 512], F32, tag="pj")
                    for c in range(NCH):
                        nc.tensor.matmul(
                            out=ps[:, 0:256], lhsT=xT_sb[:, c, 128 * t:128 * (t + 1)],
                            rhs=wv_sb[:, c, 256 * g:256 * (g + 1)],
                            start=(c == 0), stop=(c == NCH - 1),
                        )
                    nc.vector.tensor_copy(
                        out=V_sb[:, t, 4 * g:4 * (g + 1), 0:64],
                        in_=ps[:, 0:256].rearrange("p (h d) -> p h d", h=4),
                    )
                return run

            def qk_task(which, pr, qc):
                # Q or K proj for pair pr, 512-col chunk qc
                def run():
                    if qc == 0:
                        w_tiles[(which, pr)] = w_ring.tile(
                            [128, NCH, 128], F32R, tag=which,
                            name=f"w_{which}{pr}")
                        src = wq if which == "q" else wk
                        nc.sync.dma_start(
                            out=w_tiles[(which, pr)],
                            in_=src[:, 128 * pr:128 * (pr + 1)].rearrange(
                                "(c p) m -> p c m", p=128))
                        dst = qk_ring.tile([128, S], BF16, tag=which,
                                           name=f"qk_{which}{pr}")
                        if which == "q":
                            qt_tiles[pr] = dst
                        else:
                            kt_tiles[pr] = dst
                    w_sb = w_tiles[(which, pr)]
                    dst = qt_tiles[pr] if which == "q" else kt_tiles[pr]
                    bcol = pr if which == "q" else 4 + pr
                    ps = pps.tile([128, 512], F32, tag="pj")
                    for c in range(NCH):
                        nc.tensor.matmul(
                            out=ps, lhsT=w_sb[:, c, :],
                            rhs=xT_sb[:, c, 512 * qc:512 * (qc + 1)],
                            start=(c == 0), stop=(c == NCH - 1),
                        )
                    nc.vector.tensor_scalar_add(
                        out=dst[:, 512 * qc:512 * (qc + 1)], in0=ps,
                        scalar1=bqk_sb[:, bcol:bcol + 1],
                    )
                return run

            def bs_task(pr):
                # suffix sums of w*colsum256(V0) for pair pr
                def run():
                    psb = pps.tile([128, 8], F32, tag="pj")
                    for c in range(NCH):
                        nc.tensor.matmul(
                            out=psb, lhsT=wv_sb[:, c, 128 * pr:128 * (pr + 1)],
                            rhs=xsum_sb[:, c, :],
                            start=(c == 0), stop=(c == NCH - 1),
                        )
                    for hl in range(2):
                        nc.vector.memset(suf_sb[:, pr, hl, 8:9], 0.0)
                        for i in range(7, -1, -1):
                            nc.vector.scalar_tensor_tensor(
                                out=suf_sb[:, pr, hl, i:i + 1],
                                in0=psb[64 * hl:64 * hl + 64, i:i + 1],
                                scalar=W_MASK, in1=suf_sb[:, pr, hl, i + 1:i + 2],
                                op0=ALU.mult, op1=ALU.add,
                            )
                return run

            def wo_task(hc):
                def run():
                    wo_f = wof.tile([128, 8, 128], F32, tag="wof")
                    nc.sync.dma_start(
                        out=wo_f,
                        in_=wo[128 * hc:128 * (hc + 1), :].rearrange(
                            "p (d m) -> p d m", m=128))
                    nc.vector.tensor_copy(out=wo_bf[:, hc, :, :], in_=wo_f)
                return run

            # upfront: V for pairs {0,1}, Q/K for pair 0, suffix sums
            for t in range(16):
                v_task(0, t)()
            for qc in range(NT):
                qk_task("q", 0, qc)()
                qk_task("k", 0, qc)()
            for pr in range(NP):
                bs_task(pr)()

            taskq = []
            for qc in range(NT):
                taskq.append(qk_task("q", 1, qc))
                taskq.append(qk_task("k", 1, qc))
            for hc in range(NP):
                taskq.append(wo_task(hc))
            for t in range(16):
                taskq.append(v_task(1, t))
            for pr in range(2, NP):
                for qc in range(NT):
                    taskq.append(qk_task("q", pr, qc))
                    taskq.append(qk_task("k", pr, qc))

            def pop_task():
                if taskq:
                    taskq.pop(0)()

            # ---------------- attention ----------------
            for pr in range(NP):
                QT = qt_tiles[pr]
                KT = kt_tiles[pr]
                for t in range(NT):
                    q0 = 512 * t
                    # chunks: (key_offset, q_lo, q_n, mask_kind)
                    chunks = []
                    for kb in range(t):
                        chunks.append((512 * kb, q0, 512, None))
                        chunks.append((512 * kb + 256, q0, 512, None))
                    chunks.append((q0, q0, 512, "A"))
                    chunks.append((q0 + 256, q0 + 256, 256, "B"))

                    po = [pop.tile([65, 512], F32, tag="po", name=f"po{hl}")
                          for hl in range(2)]
                    ntot = len(chunks)
                    pend = []

                    def emit_scores(ci, chunks=chunks, pend=pend, QT=QT, KT=KT):
                        ko, qlo, qn, _ = chunks[ci]
                        pt = [sps.tile([128, 2, 512], F32, tag="s",
                                       name=f"pt{hl}") for hl in range(2)]
                        for ks in range(2):
                            for hl in range(2):
                                hs = slice(64 * hl, 64 * (hl + 1))
                                nc.tensor.matmul(
                                    out=pt[hl][:, ks, 0:qn],
                                    lhsT=KT[hs, ko + 128 * ks:ko + 128 * (ks + 1)],
                                    rhs=QT[hs, qlo:qlo + qn],
                                    start=True, stop=True,
                                )
                        pend.append((ci, pt))

                    def emit_exp(ci, pt, chunks=chunks):
                        ko, qlo, qn, mr = chunks[ci]
                        et = []
                        for hl in range(2):
                            if mr == "A":
                                nc.vector.scalar_tensor_tensor(
                                    out=pt[hl][:, :, 0:512],
                                    in0=pt[hl][:, :, 0:512],
                                    scalar=8e-4, in1=mmul_sb[:, :, 0:512],
                                    op0=ALU.add, op1=ALU.mult,
                                )
                            elif mr == "B":
                                nc.vector.scalar_tensor_tensor(
                                    out=pt[hl][:, :, 0:256],
                                    in0=pt[hl][:, :, 0:256],
                                    scalar=8e-4, in1=mmul_sb[:, :, 512:768],
                                    op0=ALU.add, op1=ALU.mult,
                                )
                            e = epool.tile([128, 2, 512], F32R, tag="e")
                            nc.scalar.activation(
                                out=e[:, :, 0:qn], in_=pt[hl][:, :, 0:qn],
                                func=AF.Exp, scale=0.125,
                                bias=(bias_neg[:, 0:1] if mr else 0.0),
                            )
                            et.append(e)
                        return et

                    def emit_pv(ci, et, chunks=chunks, po=po, ntot=ntot,
                                q0=q0, pr=pr):
                        ko, qlo, qn, _ = chunks[ci]
                        qrel = qlo - q0
                        for hl in range(2):
                            for ks in range(2):
                                tok = ko // 128 + ks
                                nc.tensor.matmul(
                                    out=po[hl][:, qrel:qrel + qn],
                                    lhsT=V_sb[:, tok, 2 * pr + hl, :],
                                    rhs=et[hl][:, ks, 0:qn],
                                    start=(ci == 0 and ks == 0),
                                    stop=(ci == ntot - 1 and ks == 1),
                                    skip_group_check=True,
                                )

                    # software-pipelined emission (lag 1 chunk)
                    for ci in range(ntot):
                        emit_scores(ci)
                        if len(pend) >= 2:
                            cj, pt = pend.pop(0)
                            emit_pv(cj, emit_exp(cj, pt))
                            pop_task()
                    while pend:
                        cj, pt = pend.pop(0)
                        emit_pv(cj, emit_exp(cj, pt))

                    # epilogue
                    for hl in range(2):
                        zf = misc.tile([1, 512], F32, tag="zf")
                        for qh in range(2):
                            nc.vector.tensor_scalar_add(
                                out=zf[:, 256 * qh:256 * (qh + 1)],
                                in0=po[hl][64:65, 256 * qh:256 * (qh + 1)],
                                scalar1=W_MASK * (S - 512 * t - 256 * (qh + 1)),
                            )
                        zi = misc.tile([1, 512], F32, tag="zi")
                        nc.vector.reciprocal_approx_fast(out=zi, in_=zf)
                        zr = misc.tile([1, 512], BF16, tag="zr")
                        nc.vector.tensor_copy(out=zr, in_=zi)
                        zb = sps.tile([64, 512], F32, tag="s")
                        nc.tensor.matmul(
                            out=zb, lhsT=ones_bf, rhs=zr,
                            start=True, stop=True, skip_group_check=True,
                        )
                        nm = misc.tile([64, 512], F32, tag="nm")
                        for qh in range(2):
                            nc.vector.tensor_scalar_add(
                                out=nm[:, 256 * qh:256 * (qh + 1)],
                                in0=po[hl][0:64, 256 * qh:256 * (qh + 1)],
                                scalar1=suf_sb[:, pr, hl,
                                               2 * t + 1 + qh:2 * t + 2 + qh],
                            )
                        nc.vector.tensor_mul(
                            out=O_sb[64 * hl:64 * (hl + 1), pr, q0:q0 + 512],
                            in0=nm, in1=zb,
                        )
                    pop_task()

            while taskq:
                taskq.pop(0)()

        # ---------------- output projection (partial over 8 heads) ----------
        with tc.tile_pool(name="fps", bufs=4, space="PSUM") as fps, \
             tc.tile_pool(name="fout", bufs=4) as fo_pool:
            for dc in range(8):
                for qc in range(NT):
                    ps = fps.tile([128, 512], F32)
                    for hc in range(NP):
                        nc.tensor.matmul(
                            out=ps, lhsT=wo_bf[:, hc, dc, :],
                            rhs=O_sb[:, hc, 512 * qc:512 * (qc + 1)],
                            start=(hc == 0), stop=(hc == NP - 1),
                        )
                    fo = fo_pool.tile([128, 512], F32)
                    nc.vector.tensor_scalar_add(
                        out=fo, in0=ps, scalar1=bocol_sb[:, dc:dc + 1])
                    nc.sync.dma_start(
                        out=outT[128 * dc:128 * (dc + 1), 512 * qc:512 * (qc + 1)],
                        in_=fo,
                    )
    nc.compile()
    return nc


def host_in_maps(x, Wqkv, bqkv, Wo, bo):
    x = np.asarray(x, np.float32)
    Wqkv = np.ascontiguousarray(np.asarray(Wqkv, np.float32))
    bqkv = np.asarray(bqkv, np.float32)
    Wo = np.ascontiguousarray(np.asarray(Wo, np.float32))
    bo = np.asarray(bo, np.float32)

    # triangular 256x256 mask for 2 key sub-blocks, plus all-ones pad
    kap = np.arange(128)[:, None]
    r = np.arange(256)[None, :]
    tri = np.zeros((128, 2, 256), np.float32)
    for s2 in range(2):
        tri[:, s2, :] = (128 * s2 + kap <= r)
    import ml_dtypes
    mmul = np.ones((128, 2, 768), np.float32)
    mmul[:, :, 0:256] = tri
    mmul[:, :, 512:768] = tri
    mmul = np.ascontiguousarray(mmul.astype(ml_dtypes.bfloat16))

    xTs = [np.ascontiguousarray(x[b].T) for b in range(B)]
    per_p = {}
    for p in range(2):
        cs = slice(512 * p, 512 * p + 512)
        bq = bqkv[0:D][cs]
        bk = bqkv[D:2 * D][cs]
        bv = bqkv[2 * D:][cs]
        wo_p = np.ascontiguousarray(Wo[cs, :])
        bqk = np.zeros((128, 8), np.float32)
        for pr in range(NP):
            bqk[:, pr] = bq[128 * pr:128 * (pr + 1)]
            bqk[:, 4 + pr] = bk[128 * pr:128 * (pr + 1)]
        boc = bv @ wo_p + (bo if p == 0 else 0.0)
        bocol = np.ascontiguousarray(boc.reshape(8, 128).T)
        per_p[p] = {
            "wq": np.ascontiguousarray(Wqkv[:, cs]),
            "wk": np.ascontiguousarray(Wqkv[:, D + 512 * p:D + 512 * p + 512]),
            "wv": np.ascontiguousarray(Wqkv[:, 2 * D + 512 * p:2 * D + 512 * p + 512]),
            "wo": wo_p,
            "bqk": bqk,
            "bocol": bocol,
        }

    in_maps = []
    for core in range(8):
        b, p = core // 2, core % 2
        m = {"xT": xTs[b], "mmul": mmul}
        m.update(per_p[p])
        in_maps.append(m)
    return in_maps


def assemble(results):
    out = np.zeros((B, S, D), np.float32)
    for b in range(B):
        out[b] = (results[2 * b]["outT"] + results[2 * b + 1]["outT"]).T
    return out


_CACHED = {}


def get_program():
    if "nc" not in _CACHED:
        _CACHED["nc"] = build_program()
    return _CACHED["nc"]


def kernel(x, Wqkv, bqkv, Wo, bo):
    from concourse.bass_utils import run_bass_kernel_spmd

    nc = get_program()
    in_maps = host_in_maps(x, Wqkv, bqkv, Wo, bo)
    res = run_bass_kernel_spmd(nc, in_maps, core_ids=list(range(8)))
    return assemble(res.results)


# revision 11
# speedup vs baseline: 1.2634x; 1.0998x over previous
"""Trainium2 Bass kernel for causal multi-head attention block.

Reference computation (B=4, S=2048, D=1024, H=16, HD=64, fp32):
    qkv = x @ Wqkv + bqkv; split q,k,v; per-head scaled scores;
    causal mask filled with -0.0001 (leaky, NOT -inf); softmax over all
    2048 keys; out = P @ V; out = out @ Wo + bo.

Sharding (head-split tensor parallel): core i = (batch b = i//2,
head half p = i%2). Each core computes ALL 2048 queries of its batch
for heads 8p..8p+7: QKV projections column-sharded by head, attention
device-local, output projection row-sharded (contraction over this
core's 512 head-dims) -> partial outputs. The two partials per batch
are summed at unshard time (host gather). The V bias is absorbed into
the per-core output bias: out_head = P@V0 + bv exactly (softmax rows
sum to 1), so bv contributes bv @ Wo_mine.

Leaky-mask algebra (w = exp(-1e-4)):
  - scores per 512-query tile t against key blocks 0..t; the diagonal
    block is split so the fully-masked upper 256-key piece of the
    first query half is never computed.
  - masked chunks: S' = (S + 8e-4) * M fused on PSUM (one DVE op),
    then exp(0.125*S' - 1e-4) = exp(S/8) unmasked / w masked.
  - skipped key blocks contribute w*Suf[d] to the numerator (suffix
    sums of unbiased V at 256-block granularity) and w*nskip to Z.
Z comes from a 65th all-ones V column in the PV matmul; 1/Z is
broadcast across the 64 head-dims with a rank-1 PE matmul.

Scheduling: the scores->exp->PV chain is software-pipelined (lag 1
chunk); tile epilogues are deferred into the next tile's chunk stream
so the in-order PE queue never blocks on the DVE z-chain; projection
work (Q/K per pair, V per head-group, wo convert, output projection
per query chunk) is sliced into tasks popped between attention chunks
as PE filler, with label gating for emission-order correctness. This
keeps the PE stream dense so the HAM clock stays at 2.4 GHz.
"""

import math
from contextlib import ExitStack

import numpy as np

import concourse.bass as bass
import concourse.mybir as mybir
import concourse.tile as tile
from concourse import bacc

F32 = mybir.dt.float32
F32R = mybir.dt.float32r
BF16 = mybir.dt.bfloat16
AF = mybir.ActivationFunctionType
ALU = mybir.AluOpType
AX = mybir.AxisListType

B, S, D, H, HD = 4, 2048, 1024, 16, 64
HPC = 8            # heads per core
NP = 4             # head pairs per core
NCH = D // 128     # contraction chunks
NT = 4             # 512-query tiles
W_MASK = math.exp(-1e-4)


def build_program():
    nc = bacc.Bacc(
        "TRN2",
        target_bir_lowering=False,
        debug=False,
        num_devices=8,
    )
    xT = nc.declare_dram_parameter("xT", [D, S], F32R, isOutput=False)
    xsum = nc.declare_dram_parameter("xsum", [128, NCH, 8], F32R, isOutput=False)
    wq = nc.declare_dram_parameter("wq", [D, 512], F32R, isOutput=False)
    wk = nc.declare_dram_parameter("wk", [D, 512], F32R, isOutput=False)
    wv = nc.declare_dram_parameter("wv", [D, 512], F32R, isOutput=False)
    wo = nc.declare_dram_parameter("wo", [512, D], F32, isOutput=False)
    bqk = nc.declare_dram_parameter("bqk", [128, 8], F32, isOutput=False)
    bocol = nc.declare_dram_parameter("bocol", [128, 8], F32, isOutput=False)
    mmul = nc.declare_dram_parameter("mmul", [128, 2, 768], BF16, isOutput=False)
    outT = nc.declare_dram_parameter("outT", [D, S], F32, isOutput=True)

    with tile.TileContext(nc) as tc, ExitStack() as ctx, \
         nc.allow_low_precision(reason="float32r matmul inputs are fp32 bits"):
        consts = ctx.enter_context(tc.tile_pool(name="consts", bufs=1))
        bqk_sb = consts.tile([128, 8], F32)
        nc.sync.dma_start(out=bqk_sb, in_=bqk[:])
        bocol_sb = consts.tile([128, 8], F32)
        nc.sync.dma_start(out=bocol_sb, in_=bocol[:])
        mmul_sb = consts.tile([128, 2, 768], BF16)
        nc.sync.dma_start(out=mmul_sb, in_=mmul[:])
        xsum_sb = consts.tile([128, NCH, 8], F32R)
        nc.sync.dma_start(out=xsum_sb, in_=xsum[:])
        onef = consts.tile([128, 128], F32)
        nc.vector.memset(onef, 1.0)
        ones_bf = consts.tile([1, 64], BF16)
        nc.vector.tensor_copy(out=ones_bf, in_=onef[0:1, 0:64])
        bias_neg = consts.tile([128, 1], F32)
        nc.vector.memset(bias_neg, -1e-4)

        xt_pool = ctx.enter_context(tc.tile_pool(name="xt", bufs=1))
        xT_sb = xt_pool.tile([128, NCH, S], F32R)
        for c in range(NCH):
            nc.sync.dma_start(out=xT_sb[:, c, :], in_=xT[128 * c:128 * (c + 1), :])

        wv_pool = ctx.enter_context(tc.tile_pool(name="wvp", bufs=1))
        wv_sb = wv_pool.tile([128, NCH, 512], F32R)
        nc.sync.dma_start(
            out=wv_sb, in_=wv[:].rearrange("(c p) m -> p c m", p=128))

        # persistent attention-side tensors
        big = ctx.enter_context(tc.tile_pool(name="big", bufs=1))
        V_sb = big.tile([128, 16, HPC, 65], F32R)  # [key sub, tok blk, head, d+1]
        O_sb = big.tile([128, NP, S], BF16)        # [2 heads x 64, chunk(=pair), q]
        suf_sb = big.tile([64, NP, 2, 9], F32)     # [d, pair, head, 256-block idx]
        wo_bf = big.tile([128, NP, 8, 128], BF16)
        nc.vector.tensor_copy(
            out=V_sb[:, :, :, 64],
            in_=onef.rearrange("p (a b) -> p a b", a=16)[:, :, 0:8])

        qk_ring = ctx.enter_context(tc.tile_pool(name="qkr", bufs=2))
        w_ring = ctx.enter_context(tc.tile_pool(name="wr", bufs=2))
        wof = ctx.enter_context(tc.tile_pool(name="wof", bufs=1))

        qt_tiles = {}
        kt_tiles = {}
        w_tiles = {}
        misc_holder = {}

        # ================= task machinery =================
        # Each task: (label, closure(pool)). Tasks emit PE work into the
        # given PSUM pool via tag "pj". Label gating guarantees emission-
        # order correctness; surplus tasks are popped one per chunk as PE
        # filler so the PE stream stays dense.
        def v_task(g, t):
            def run(pool):
                ps = pool.tile([128, 512], F32, tag="pj", name="psv", bufs=1)
                for c in range(NCH):
                    nc.tensor.matmul(
                        out=ps[:, 0:256], lhsT=xT_sb[:, c, 128 * t:128 * (t + 1)],
                        rhs=wv_sb[:, c, 256 * g:256 * (g + 1)],
                        start=(c == 0), stop=(c == NCH - 1),
                    )
                nc.vector.tensor_copy(
                    out=V_sb[:, t, 4 * g:4 * (g + 1), 0:64],
                    in_=ps[:, 0:256].rearrange("p (h d) -> p h d", h=4),
                )
            return (("v", g, t), run)

        def qk_task(which, pr, qc):
            def run(pool):
                if qc == 0:
                    w_tiles[(which, pr)] = w_ring.tile(
                        [128, NCH, 128], F32R, tag=which,
                        name=f"w_{which}{pr}")
                    src = wq if which == "q" else wk
                    nc.sync.dma_start(
                        out=w_tiles[(which, pr)],
                        in_=src[:, 128 * pr:128 * (pr + 1)].rearrange(
                            "(c p) m -> p c m", p=128))
                    dst = qk_ring.tile([128, S], BF16, tag=which,
                                       name=f"qk_{which}{pr}")
                    if which == "q":
                        qt_tiles[pr] = dst
                    else:
                        kt_tiles[pr] = dst
                w_sb = w_tiles[(which, pr)]
                dst = qt_tiles[pr] if which == "q" else kt_tiles[pr]
                bcol = pr if which == "q" else 4 + pr
                ps = pool.tile([128, 512], F32, tag="pj", name="psqk", bufs=1)
                for c in range(NCH):
                    nc.tensor.matmul(
                        out=ps, lhsT=w_sb[:, c, :],
                        rhs=xT_sb[:, c, 512 * qc:512 * (qc + 1)],
                        start=(c == 0), stop=(c == NCH - 1),
                    )
                nc.vector.tensor_scalar_add(
                    out=dst[:, 512 * qc:512 * (qc + 1)], in0=ps,
                    scalar1=bqk_sb[:, bcol:bcol + 1],
                )
            return ((which, pr, qc), run)

        def bs_task(pr):
            def run(pool):
                psb = pool.tile([128, 8], F32, tag="pj", name="psb", bufs=1)
                for c in range(NCH):
                    nc.tensor.matmul(
                        out=psb, lhsT=wv_sb[:, c, 128 * pr:128 * (pr + 1)],
                        rhs=xsum_sb[:, c, :],
                        start=(c == 0), stop=(c == NCH - 1),
                    )
                for hl in range(2):
                    nc.vector.memset(suf_sb[:, pr, hl, 8:9], 0.0)
                    for i in range(7, -1, -1):
                        nc.vector.scalar_tensor_tensor(
                            out=suf_sb[:, pr, hl, i:i + 1],
                            in0=psb[64 * hl:64 * hl + 64, i:i + 1],
                            scalar=W_MASK, in1=suf_sb[:, pr, hl, i + 1:i + 2],
                            op0=ALU.mult, op1=ALU.add,
                        )
            return (("bs", pr), run)

        def wo_task(hc):
            def run(pool):
                wo_f = wof.tile([128, 8, 128], F32, tag="wof", name="wo_f")
                nc.sync.dma_start(
                    out=wo_f,
                    in_=wo[128 * hc:128 * (hc + 1), :].rearrange(
                        "p (d m) -> p d m", m=128))
                nc.vector.tensor_copy(out=wo_bf[:, hc, :, :], in_=wo_f)
            return (("wo", hc), run)

        def oproj_task(dc, qc):
            def run(pool):
                ps = pool.tile([128, 512], F32, tag="pj", name="psop", bufs=1)
                for hc in range(NP):
                    nc.tensor.matmul(
                        out=ps, lhsT=wo_bf[:, hc, dc, :],
                        rhs=O_sb[:, hc, 512 * qc:512 * (qc + 1)],
                        start=(hc == 0), stop=(hc == NP - 1),
                    )
                fo = misc_holder["misc"].tile([128, 512], F32, tag="fo")
                nc.vector.tensor_scalar_add(
                    out=fo, in0=ps, scalar1=bocol_sb[:, dc:dc + 1])
                nc.sync.dma_start(
                    out=outT[128 * dc:128 * (dc + 1), 512 * qc:512 * (qc + 1)],
                    in_=fo,
                )
            return (("op", dc, qc), run)

        taskq = []
        done_labels = set()

        def pop_one(pool):
            if taskq:
                label, run = taskq.pop(0)
                run(pool)
                done_labels.add(label)

        def ensure(labels, pool):
            while taskq and not all(l in done_labels for l in labels):
                pop_one(pool)

        # ---------------- upfront minimal (own PSUM scope) ----------------
        with tc.tile_pool(name="ups", bufs=4, space="PSUM") as ups:
            for t in range(4):
                v_task(0, t)[1](ups)
            qk_task("q", 0, 0)[1](ups)
            qk_task("k", 0, 0)[1](ups)
        done_labels.update({("v", 0, t) for t in range(4)})
        done_labels.update({("q", 0, 0), ("k", 0, 0)})

        # remaining work as ordered tasks
        for pr in range(NP):
            taskq.append(bs_task(pr))
        for qc in range(1, NT):
            taskq.append(qk_task("k", 0, qc))
            taskq.append(qk_task("q", 0, qc))
            for t in range(4 * qc, 4 * qc + 4):
                taskq.append(v_task(0, t))
        for qc in range(NT):
            taskq.append(qk_task("q", 1, qc))
            taskq.append(qk_task("k", 1, qc))
        for hc in range(NP):
            taskq.append(wo_task(hc))
        for t in range(16):
            taskq.append(v_task(1, t))
        for pr in range(2, NP):
            for qc in range(NT):
                taskq.append(qk_task("q", pr, qc))
                taskq.append(qk_task("k", pr, qc))

        # ---------------- attention ----------------
        with tc.tile_pool(name="sps", bufs=2, space="PSUM") as sps, \
             tc.tile_pool(name="pop", bufs=3, space="PSUM") as pop, \
             tc.tile_pool(name="epool", bufs=3) as epool, \
             tc.tile_pool(name="misc", bufs=2) as misc:
            misc_holder["misc"] = misc

            deferred = []   # epilogue-PE + nm/ot closures of previous tile

            def flush_deferred():
                while deferred:
                    deferred.pop(0)()

            for pr in range(NP):
                ensure([("q", pr, 0), ("k", pr, 0)]
                       + [("v", pr // 2, t) for t in range(4)], sps)
                QT = qt_tiles[pr]
                KT = kt_tiles[pr]
                for t in range(NT):
                    ensure([("q", pr, t)] + [("k", pr, kc) for kc in range(t + 1)]
                           + [("v", pr // 2, tb) for tb in range(4 * t + 4)]
                           + [("bs", pr)], sps)
                    q0 = 512 * t
                    chunks = []
                    for kb in range(t):
                        for s2 in range(4):
                            chunks.append((512 * kb + 128 * s2, q0, 512, None))
                    for s2 in range(2):
                        chunks.append((q0 + 128 * s2, q0, 512, ("A", s2)))
                    for s2 in range(2):
                        chunks.append((q0 + 256 + 128 * s2, q0 + 256, 256,
                                       ("B", s2)))

                    po = [pop.tile([65, 512], F32, tag="po", name=f"po{hl}")
                          for hl in range(2)]
                    ntot = len(chunks)
                    pend = []

                    def emit_scores(ci, chunks=chunks, pend=pend, QT=QT, KT=KT):
                        ko, qlo, qn, _ = chunks[ci]
                        pt = sps.tile([128, 2, 512], F32, tag="s", name="pt")
                        for hl in range(2):
                            hs = slice(64 * hl, 64 * (hl + 1))
                            nc.tensor.matmul(
                                out=pt[:, hl, 0:qn],
                                lhsT=KT[hs, ko:ko + 128],
                                rhs=QT[hs, qlo:qlo + qn],
                                start=True, stop=True,
                            )
                        pend.append((ci, pt))

                    def emit_exp(ci, pt, chunks=chunks):
                        ko, qlo, qn, mr = chunks[ci]
                        if mr is not None:
                            kind, s2 = mr
                            mslice = (mmul_sb[:, s2:s2 + 1, 0:512]
                                      if kind == "A"
                                      else mmul_sb[:, s2:s2 + 1, 512:768])
                            nc.vector.scalar_tensor_tensor(
                                out=pt[:, :, 0:qn], in0=pt[:, :, 0:qn],
                                scalar=8e-4,
                                in1=mslice.broadcast_to([128, 2, qn]),
                                op0=ALU.add, op1=ALU.mult,
                            )
                        e = epool.tile([128, 2, 512], F32R, tag="e")
                        nc.scalar.activation(
                            out=e[:, :, 0:qn], in_=pt[:, :, 0:qn],
                            func=AF.Exp, scale=0.125,
                            bias=(bias_neg[:, 0:1] if mr else 0.0),
                        )
                        return e

                    def emit_pv(ci, e, chunks=chunks, po=po, ntot=ntot,
                                q0=q0, pr=pr):
                        ko, qlo, qn, _ = chunks[ci]
                        qrel = qlo - q0
                        for hl in range(2):
                            nc.tensor.matmul(
                                out=po[hl][:, qrel:qrel + qn],
                                lhsT=V_sb[:, ko // 128, 2 * pr + hl, :],
                                rhs=e[:, hl, 0:qn],
                                start=(ci == 0),
                                stop=(ci == ntot - 1),
                                skip_group_check=True,
                            )

                    # software-pipelined emission (lag 1 chunk); previous
                    # tile's deferred epilogue flushes before our first PV
                    # (its po buffers are about to be reused)
                    for ci in range(ntot):
                        emit_scores(ci)
                        if len(pend) >= 2:
                            cj, pt = pend.pop(0)
                            e = emit_exp(cj, pt)
                            if cj == 0:
                                flush_deferred()
                            emit_pv(cj, e)
                            if cj != 0:
                                pop_one(sps)
                    while pend:
                        cj, pt = pend.pop(0)
                        emit_pv(cj, emit_exp(cj, pt))

                    # z-chain now (DVE only); PE parts + nm/ot deferred
                    zrow = misc.tile([1, 1024], F32, tag="zrow", bufs=1)
                    zr = misc.tile([1, 1024], BF16, tag="zr", bufs=1)
                    for hl in range(2):
                        for qh in range(2):
                            nc.vector.tensor_scalar_add(
                                out=zrow[:, 512 * hl + 256 * qh:
                                         512 * hl + 256 * (qh + 1)],
                                in0=po[hl][64:65, 256 * qh:256 * (qh + 1)],
                                scalar1=W_MASK * (S - 512 * t - 256 * (qh + 1)),
                            )
                    nc.vector.reciprocal_approx_fast(out=zrow, in_=zrow)
                    nc.vector.tensor_copy(out=zr, in_=zrow)

                    def late(po=po, zr=zr, pr=pr, t=t, q0=q0):
                        for hl in range(2):
                            zb = sps.tile([64, 512], F32, tag="s", name="zb")
                            nc.tensor.matmul(
                                out=zb, lhsT=ones_bf,
                                rhs=zr[0:1, 512 * hl:512 * (hl + 1)],
                                start=True, stop=True, skip_group_check=True,
                            )
                            nm = misc.tile([64, 512], F32, tag="nm")
                            for qh in range(2):
                                nc.vector.tensor_scalar_add(
                                    out=nm[:, 256 * qh:256 * (qh + 1)],
                                    in0=po[hl][0:64, 256 * qh:256 * (qh + 1)],
                                    scalar1=suf_sb[:, pr, hl,
                                                   2 * t + 1 + qh:2 * t + 2 + qh],
                                )
                            nc.vector.tensor_mul(
                                out=O_sb[64 * hl:64 * (hl + 1), pr, q0:q0 + 512],
                                in0=nm, in1=zb,
                            )
                    deferred.append(late)
                    if pr == NP - 1:
                        # output projection for query chunk t becomes
                        # available once this (last) pair's tile t is done
                        def oq(t=t):
                            for dc in range(8):
                                taskq.append(oproj_task(dc, t))
                        deferred.append(oq)

            flush_deferred()
            while taskq:
                pop_one(sps)
    nc.compile()
    return nc


def host_in_maps(x, Wqkv, bqkv, Wo, bo):
    import ml_dtypes
    x = np.asarray(x, np.float32)
    Wqkv = np.ascontiguousarray(np.asarray(Wqkv, np.float32))
    bqkv = np.asarray(bqkv, np.float32)
    Wo = np.ascontiguousarray(np.asarray(Wo, np.float32))
    bo = np.asarray(bo, np.float32)

    # triangular 256x256 mask for 2 key sub-blocks, plus all-ones pad
    kap = np.arange(128)[:, None]
    r = np.arange(256)[None, :]
    tri = np.zeros((128, 2, 256), np.float32)
    for s2 in range(2):
        tri[:, s2, :] = (128 * s2 + kap <= r)
    mmul = np.ones((128, 2, 768), np.float32)
    mmul[:, :, 0:256] = tri
    mmul[:, :, 512:768] = tri
    mmul = np.ascontiguousarray(mmul.astype(ml_dtypes.bfloat16))

    xTs = []
    xsums = []
    for b in range(B):
        xt = np.ascontiguousarray(x[b].T)                # [D, S]
        xTs.append(xt)
        xs = xt.reshape(NCH, 128, 8, 256).sum(axis=3)    # [c, p, blk]
        xsums.append(np.ascontiguousarray(xs.transpose(1, 0, 2)))
    per_p = {}
    for p in range(2):
        cs = slice(512 * p, 512 * p + 512)
        bq = bqkv[0:D][cs]
        bk = bqkv[D:2 * D][cs]
        bv = bqkv[2 * D:][cs]
        wo_p = np.ascontiguousarray(Wo[cs, :])
        bqk = np.zeros((128, 8), np.float32)
        for pr in range(NP):
            bqk[:, pr] = bq[128 * pr:128 * (pr + 1)]
            bqk[:, 4 + pr] = bk[128 * pr:128 * (pr + 1)]
        boc = bv @ wo_p + (bo if p == 0 else 0.0)
        bocol = np.ascontiguousarray(boc.reshape(8, 128).T)
        per_p[p] = {
            "wq": np.ascontiguousarray(Wqkv[:, cs]),
            "wk": np.ascontiguousarray(Wqkv[:, D + 512 * p:D + 512 * p + 512]),
            "wv": np.ascontiguousarray(Wqkv[:, 2 * D + 512 * p:2 * D + 512 * p + 512]),
            "wo": wo_p,
            "bqk": bqk,
            "bocol": bocol,
        }

    in_maps = []
    for core in range(8):
        b, p = core // 2, core % 2
        m = {"xT": xTs[b], "xsum": xsums[b], "mmul": mmul}
        m.update(per_p[p])
        in_maps.append(m)
    return in_maps


def assemble(results):
    out = np.zeros((B, S, D), np.float32)
    for b in range(B):
        out[b] = (results[2 * b]["outT"] + results[2 * b + 1]["outT"]).T
    return out


_CACHED = {}


def get_program():
    if "nc" not in _CACHED:
        _CACHED["nc"] = build_program()
    return _CACHED["nc"]


def kernel(x, Wqkv, bqkv, Wo, bo):
    from concourse.bass_utils import run_bass_kernel_spmd

    nc = get_program()
    in_maps = host_in_maps(x, Wqkv, bqkv, Wo, bo)
    res = run_bass_kernel_spmd(nc, in_maps, core_ids=list(range(8)))
    return assemble(res.results)


# revision 13
# speedup vs baseline: 1.4508x; 1.1483x over previous
"""Trainium2 Bass kernel for causal multi-head attention block.

Reference computation (B=4, S=2048, D=1024, H=16, HD=64, fp32):
    qkv = x @ Wqkv + bqkv; split q,k,v; per-head scaled scores;
    causal mask filled with -0.0001 (leaky, NOT -inf); softmax over all
    2048 keys; out = P @ V; out = out @ Wo + bo.

Sharding (head-split tensor parallel): core i = (batch b = i//2,
head half p = i%2). Each core computes ALL 2048 queries of its batch
for heads 8p..8p+7: QKV projections column-sharded by head, attention
device-local, output projection row-sharded (contraction over this
core's 512 head-dims) -> partial outputs. The two partials per batch
are summed at unshard time (host gather). The V bias is absorbed into
the per-core output bias: out_head = P@V0 + bv exactly (softmax rows
sum to 1), so bv contributes bv @ Wo_mine.

Leaky-mask algebra (w = exp(-1e-4)):
  - scores per 512-query tile t against key blocks 0..t; the diagonal
    block is split so the fully-masked upper 256-key piece of the
    first query half is never computed.
  - masked chunks: S' = (S + 8e-4) * M fused on PSUM (one DVE op),
    then exp(0.125*S' - 1e-4) = exp(S/8) unmasked / w masked.
  - skipped key blocks contribute w*Suf[d] to the numerator (suffix
    sums of unbiased V at 256-block granularity) and w*nskip to Z.
Z comes from a 65th all-ones V column in the PV matmul; 1/Z is
broadcast across the 64 head-dims with a rank-1 PE matmul.

Scheduling: the scores->exp->PV chain is software-pipelined (lag 1
chunk); tile epilogues are deferred into the next tile's chunk stream
so the in-order PE queue never blocks on the DVE z-chain; projection
work (Q/K per pair, V per head-group, wo convert, output projection
per query chunk) is sliced into tasks popped between attention chunks
as PE filler, with label gating for emission-order correctness. This
keeps the PE stream dense so the HAM clock stays at 2.4 GHz.
"""

import math
from contextlib import ExitStack

import numpy as np

import concourse.bass as bass
import concourse.mybir as mybir
import concourse.tile as tile
from concourse import bacc

F32 = mybir.dt.float32
F32R = mybir.dt.float32r
BF16 = mybir.dt.bfloat16
AF = mybir.ActivationFunctionType
ALU = mybir.AluOpType
AX = mybir.AxisListType

B, S, D, H, HD = 4, 2048, 1024, 16, 64
HPC = 8            # heads per core
NP = 4             # head pairs per core
NCH = D // 128     # contraction chunks
NT = 4             # 512-query tiles
W_MASK = math.exp(-1e-4)


def build_program():
    nc = bacc.Bacc(
        "TRN2",
        target_bir_lowering=False,
        debug=False,
        num_devices=8,
    )
    xT = nc.declare_dram_parameter("xT", [D, S], BF16, isOutput=False)
    xsum = nc.declare_dram_parameter("xsum", [128, NCH, 8], BF16, isOutput=False)
    wq = nc.declare_dram_parameter("wq", [D, 512], BF16, isOutput=False)
    wk = nc.declare_dram_parameter("wk", [D, 512], BF16, isOutput=False)
    wv = nc.declare_dram_parameter("wv", [D, 512], BF16, isOutput=False)
    wo = nc.declare_dram_parameter("wo", [512, D], F32, isOutput=False)
    bqk = nc.declare_dram_parameter("bqk", [128, 8], F32, isOutput=False)
    bocol = nc.declare_dram_parameter("bocol", [128, 8], F32, isOutput=False)
    mmul = nc.declare_dram_parameter("mmul", [128, 2, 768], BF16, isOutput=False)
    outT = nc.declare_dram_parameter("outT", [D, S], F32, isOutput=True)

    with tile.TileContext(nc) as tc, ExitStack() as ctx, \
         nc.allow_low_precision(reason="float32r matmul inputs are fp32 bits"):
        consts = ctx.enter_context(tc.tile_pool(name="consts", bufs=1))
        bqk_sb = consts.tile([128, 8], F32)
        nc.sync.dma_start(out=bqk_sb, in_=bqk[:])
        bocol_sb = consts.tile([128, 8], F32)
        nc.sync.dma_start(out=bocol_sb, in_=bocol[:])
        mmul_sb = consts.tile([128, 2, 768], BF16)
        nc.sync.dma_start(out=mmul_sb, in_=mmul[:])
        xsum_sb = consts.tile([128, NCH, 8], BF16)
        nc.sync.dma_start(out=xsum_sb, in_=xsum[:])
        onef = consts.tile([128, 128], F32)
        nc.vector.memset(onef, 1.0)
        ones_fr = consts.tile([1, 64], F32R)
        nc.vector.tensor_copy(out=ones_fr, in_=onef[0:1, 0:64])
        bias_neg = consts.tile([128, 1], F32)
        nc.vector.memset(bias_neg, -1e-4)

        wv_pool = ctx.enter_context(tc.tile_pool(name="wvp", bufs=1))
        wv_sb = wv_pool.tile([128, NCH, 512], BF16)
        nc.sync.dma_start(
            out=wv_sb, in_=wv[:].rearrange("(c p) m -> p c m", p=128))

        xt_pool = ctx.enter_context(tc.tile_pool(name="xt", bufs=1))
        xT_sb = xt_pool.tile([128, NCH, S], BF16)
        for c in range(NCH):
            nc.sync.dma_start(out=xT_sb[:, c, :], in_=xT[128 * c:128 * (c + 1), :])

        # persistent attention-side tensors
        big = ctx.enter_context(tc.tile_pool(name="big", bufs=1))
        V_sb = big.tile([128, 16, HPC, 65], F32R)  # [key sub, tok blk, head, d+1]
        O_sb = big.tile([128, NP, S], BF16)        # [2 heads x 64, chunk(=pair), q]
        suf_sb = big.tile([64, NP, 2, 9], F32)     # [d, pair, head, 256-block idx]
        wo_bf = big.tile([128, NP, 8, 128], BF16)
        nc.vector.tensor_copy(
            out=V_sb[:, :, :, 64],
            in_=onef.rearrange("p (a b) -> p a b", a=16)[:, :, 0:8])

        qk_ring = ctx.enter_context(tc.tile_pool(name="qkr", bufs=2))
        w_ring = ctx.enter_context(tc.tile_pool(name="wr", bufs=2))
        wof = ctx.enter_context(tc.tile_pool(name="wof", bufs=1))

        qt_tiles = {}
        kt_tiles = {}
        w_tiles = {}
        misc_holder = {}

        # ================= task machinery =================
        # Each task: (label, closure(pool)). Tasks emit PE work into the
        # given PSUM pool via tag "pj". Label gating guarantees emission-
        # order correctness; surplus tasks are popped one per chunk as PE
        # filler so the PE stream stays dense.
        def v_task(g, t):
            def run(pool):
                ps = pool.tile([128, 512], F32, tag="pj", name="psv", bufs=pool._pjbufs)
                for c in range(NCH):
                    nc.tensor.matmul(
                        out=ps[:, 0:256], lhsT=xT_sb[:, c, 128 * t:128 * (t + 1)],
                        rhs=wv_sb[:, c, 256 * g:256 * (g + 1)],
                        start=(c == 0), stop=(c == NCH - 1),
                    )
                nc.vector.tensor_copy(
                    out=V_sb[:, t, 4 * g:4 * (g + 1), 0:64],
                    in_=ps[:, 0:256].rearrange("p (h d) -> p h d", h=4),
                )
            return (("v", g, t), run)

        def qk_task(which, pr, qc):
            def run(pool):
                if qc == 0:
                    w_tiles[(which, pr)] = w_ring.tile(
                        [128, NCH, 128], BF16, tag=which,
                        name=f"w_{which}{pr}")
                    src = wq if which == "q" else wk
                    nc.sync.dma_start(
                        out=w_tiles[(which, pr)],
                        in_=src[:, 128 * pr:128 * (pr + 1)].rearrange(
                            "(c p) m -> p c m", p=128))
                    dst = qk_ring.tile([128, S], BF16, tag=which,
                                       name=f"qk_{which}{pr}")
                    if which == "q":
                        qt_tiles[pr] = dst
                    else:
                        kt_tiles[pr] = dst
                w_sb = w_tiles[(which, pr)]
                dst = qt_tiles[pr] if which == "q" else kt_tiles[pr]
                bcol = pr if which == "q" else 4 + pr
                ps = pool.tile([128, 512], F32, tag="pj", name="psqk", bufs=pool._pjbufs)
                for c in range(NCH):
                    nc.tensor.matmul(
                        out=ps, lhsT=w_sb[:, c, :],
                        rhs=xT_sb[:, c, 512 * qc:512 * (qc + 1)],
                        start=(c == 0), stop=(c == NCH - 1),
                    )
                nc.vector.tensor_scalar_add(
                    out=dst[:, 512 * qc:512 * (qc + 1)], in0=ps,
                    scalar1=bqk_sb[:, bcol:bcol + 1],
                )
            return ((which, pr, qc), run)

        def bs_task(pr):
            def run(pool):
                psb = pool.tile([128, 8], F32, tag="pj", name="psb", bufs=pool._pjbufs)
                for c in range(NCH):
                    nc.tensor.matmul(
                        out=psb, lhsT=wv_sb[:, c, 128 * pr:128 * (pr + 1)],
                        rhs=xsum_sb[:, c, :],
                        start=(c == 0), stop=(c == NCH - 1),
                    )
                for hl in range(2):
                    nc.vector.memset(suf_sb[:, pr, hl, 8:9], 0.0)
                    for i in range(7, -1, -1):
                        nc.vector.scalar_tensor_tensor(
                            out=suf_sb[:, pr, hl, i:i + 1],
                            in0=psb[64 * hl:64 * hl + 64, i:i + 1],
                            scalar=W_MASK, in1=suf_sb[:, pr, hl, i + 1:i + 2],
                            op0=ALU.mult, op1=ALU.add,
                        )
            return (("bs", pr), run)

        def wo_task(hc):
            def run(pool):
                wo_f = wof.tile([128, 8, 128], F32, tag="wof", name="wo_f")
                nc.sync.dma_start(
                    out=wo_f,
                    in_=wo[128 * hc:128 * (hc + 1), :].rearrange(
                        "p (d m) -> p d m", m=128))
                nc.vector.tensor_copy(out=wo_bf[:, hc, :, :], in_=wo_f)
            return (("wo", hc), run)

        def oproj_task(dc, qc):
            def run(pool):
                ps = pool.tile([128, 512], F32, tag="pj", name="psop", bufs=pool._pjbufs)
                for hc in range(NP):
                    nc.tensor.matmul(
                        out=ps, lhsT=wo_bf[:, hc, dc, :],
                        rhs=O_sb[:, hc, 512 * qc:512 * (qc + 1)],
                        start=(hc == 0), stop=(hc == NP - 1),
                    )
                fo = misc_holder["misc"].tile([128, 512], F32, tag="fo")
                nc.vector.tensor_scalar_add(
                    out=fo, in0=ps, scalar1=bocol_sb[:, dc:dc + 1])
                nc.sync.dma_start(
                    out=outT[128 * dc:128 * (dc + 1), 512 * qc:512 * (qc + 1)],
                    in_=fo,
                )
            return (("op", dc, qc), run)

        taskq = []
        done_labels = set()

        def pop_one(pool):
            if taskq:
                label, run = taskq.pop(0)
                run(pool)
                done_labels.add(label)

        def ensure(labels, pool):
            while taskq and not all(l in done_labels for l in labels):
                pop_one(pool)

        # ---------------- upfront minimal (own PSUM scope) ----------------
        with tc.tile_pool(name="ups", bufs=4, space="PSUM") as ups:
            ups._pjbufs = 4
            for t in range(4):
                v_task(0, t)[1](ups)
            qk_task("q", 0, 0)[1](ups)
            qk_task("k", 0, 0)[1](ups)
        done_labels.update({("v", 0, t) for t in range(4)})
        done_labels.update({("q", 0, 0), ("k", 0, 0)})

        # remaining work as ordered tasks
        for pr in range(NP):
            taskq.append(bs_task(pr))
        for qc in range(1, NT):
            taskq.append(qk_task("k", 0, qc))
            taskq.append(qk_task("q", 0, qc))
            for t in range(4 * qc, 4 * qc + 4):
                taskq.append(v_task(0, t))
        for qc in range(NT):
            taskq.append(qk_task("q", 1, qc))
            taskq.append(qk_task("k", 1, qc))
        for hc in range(NP):
            taskq.append(wo_task(hc))
        for t in range(16):
            taskq.append(v_task(1, t))
        for pr in range(2, NP):
            for qc in range(NT):
                taskq.append(qk_task("q", pr, qc))
                taskq.append(qk_task("k", pr, qc))

        # ---------------- attention ----------------
        with tc.tile_pool(name="sps", bufs=2, space="PSUM") as sps, \
             tc.tile_pool(name="pop", bufs=3, space="PSUM") as pop, \
             tc.tile_pool(name="epool", bufs=4) as epool, \
             tc.tile_pool(name="misc", bufs=2) as misc:
            sps._pjbufs = 1
            misc_holder["misc"] = misc

            deferred = []   # epilogue-PE + nm/ot closures of previous tile

            def flush_deferred():
                while deferred:
                    deferred.pop(0)()

            for pr in range(NP):
                ensure([("q", pr, 0), ("k", pr, 0)]
                       + [("v", pr // 2, t) for t in range(4)], sps)
                QT = qt_tiles[pr]
                KT = kt_tiles[pr]
                for t in range(NT):
                    ensure([("q", pr, t)] + [("k", pr, kc) for kc in range(t + 1)]
                           + [("v", pr // 2, tb) for tb in range(4 * t + 4)]
                           + [("bs", pr)], sps)
                    q0 = 512 * t
                    chunks = []
                    for kb in range(t):
                        for s2 in range(4):
                            chunks.append((512 * kb + 128 * s2, q0, 512, None))
                    for s2 in range(2):
                        chunks.append((q0 + 128 * s2, q0, 512, ("A", s2)))
                    for s2 in range(2):
                        chunks.append((q0 + 256 + 128 * s2, q0 + 256, 256,
                                       ("B", s2)))

                    po = [pop.tile([65, 512], F32, tag="po", name=f"po{hl}")
                          for hl in range(2)]
                    ntot = len(chunks)
                    pend = []

                    def emit_scores(ci, chunks=chunks, pend=pend, QT=QT, KT=KT):
                        ko, qlo, qn, _ = chunks[ci]
                        pt = sps.tile([128, 2, 512], F32, tag="s", name="pt")
                        for hl in range(2):
                            hs = slice(64 * hl, 64 * (hl + 1))
                            nc.tensor.matmul(
                                out=pt[:, hl, 0:qn],
                                lhsT=KT[hs, ko:ko + 128],
                                rhs=QT[hs, qlo:qlo + qn],
                                start=True, stop=True,
                            )
                        pend.append((ci, pt))

                    def emit_exp(ci, pt, chunks=chunks):
                        # uniform bias -1e-4 scales every softmax term by w
                        # (cancels in the normalization); masked entries then
                        # come out as w (vs exact w^2) -- a 1e-4 relative
                        # perturbation of those weights, far below tolerance.
                        ko, qlo, qn, mr = chunks[ci]
                        if mr is not None:
                            _, s2 = mr
                            nc.vector.tensor_mul(
                                out=pt[:, :, 0:256], in0=pt[:, :, 0:256],
                                in1=mmul_sb[:, s2:s2 + 1, 0:256].broadcast_to(
                                    [128, 2, 256]),
                            )
                        e = epool.tile([128, 2, 512], F32R, tag="e")
                        nc.scalar.activation(
                            out=e[:, :, 0:qn], in_=pt[:, :, 0:qn],
                            func=AF.Exp, scale=0.125, bias=bias_neg[:, 0:1],
                        )
                        return e

                    def emit_pv(ci, e, chunks=chunks, po=po, ntot=ntot,
                                q0=q0, pr=pr):
                        ko, qlo, qn, _ = chunks[ci]
                        qrel = qlo - q0
                        for hl in range(2):
                            nc.tensor.matmul(
                                out=po[hl][:, qrel:qrel + qn],
                                lhsT=V_sb[:, ko // 128, 2 * pr + hl, :],
                                rhs=e[:, hl, 0:qn],
                                start=(ci == 0),
                                stop=(ci == ntot - 1),
                                skip_group_check=True,
                            )

                    # software-pipelined emission (lag 1 chunk); previous
                    # tile's deferred epilogue flushes before our first PV
                    # (its po buffers are about to be reused)
                    for ci in range(ntot):
                        emit_scores(ci)
                        if len(pend) >= 2:
                            cj, pt = pend.pop(0)
                            e = emit_exp(cj, pt)
                            if cj == 0:
                                flush_deferred()
                            emit_pv(cj, e)
                            if cj != 0:
                                pop_one(sps)
                    while pend:
                        cj, pt = pend.pop(0)
                        emit_pv(cj, emit_exp(cj, pt))

                    # z-chain now (DVE only); PE parts + nm/ot deferred
                    zrow = misc.tile([1, 1024], F32, tag="zrow", bufs=2)
                    for hl in range(2):
                        for qh in range(2):
                            nc.vector.tensor_scalar_add(
                                out=zrow[:, 512 * hl + 256 * qh:
                                         512 * hl + 256 * (qh + 1)],
                                in0=po[hl][64:65, 256 * qh:256 * (qh + 1)],
                                scalar1=W_MASK * (S - 512 * t - 256 * (qh + 1)),
                            )
                    nc.vector.reciprocal_approx_fast(out=zrow, in_=zrow)
                    zr = misc.tile([1, 1024], F32R, tag="zr", bufs=2)
                    nc.vector.tensor_copy(out=zr, in_=zrow)

                    def late(po=po, zr=zr, pr=pr, t=t, q0=q0):
                        for hl in range(2):
                            zb = sps.tile([64, 512], F32, tag="s", name="zb")
                            nc.tensor.matmul(
                                out=zb, lhsT=ones_fr,
                                rhs=zr[0:1, 512 * hl:512 * (hl + 1)],
                                start=True, stop=True, skip_group_check=True,
                            )
                            nm = misc.tile([64, 512], F32, tag="nm")
                            for qh in range(2):
                                nc.vector.tensor_scalar_add(
                                    out=nm[:, 256 * qh:256 * (qh + 1)],
                                    in0=po[hl][0:64, 256 * qh:256 * (qh + 1)],
                                    scalar1=suf_sb[:, pr, hl,
                                                   2 * t + 1 + qh:2 * t + 2 + qh],
                                )
                            nc.vector.tensor_mul(
                                out=O_sb[64 * hl:64 * (hl + 1), pr, q0:q0 + 512],
                                in0=nm, in1=zb,
                            )
                    deferred.append(late)
                    if pr == NP - 1:
                        # output projection for query chunk t becomes
                        # available once this (last) pair's tile t is done
                        def oq(t=t):
                            for dc in range(8):
                                taskq.append(oproj_task(dc, t))
                        deferred.append(oq)

            flush_deferred()
            while taskq:
                pop_one(sps)
    nc.compile()
    return nc


def host_in_maps(x, Wqkv, bqkv, Wo, bo):
    import ml_dtypes
    x = np.asarray(x, np.float32)
    Wqkv = np.ascontiguousarray(np.asarray(Wqkv, np.float32))
    bqkv = np.asarray(bqkv, np.float32)
    Wo = np.ascontiguousarray(np.asarray(Wo, np.float32))
    bo = np.asarray(bo, np.float32)

    # triangular 256x256 mask for 2 key sub-blocks, plus all-ones pad
    kap = np.arange(128)[:, None]
    r = np.arange(256)[None, :]
    tri = np.zeros((128, 2, 256), np.float32)
    for s2 in range(2):
        tri[:, s2, :] = (128 * s2 + kap <= r)
    mmul = np.ones((128, 2, 768), np.float32)
    mmul[:, :, 0:256] = tri
    mmul[:, :, 512:768] = tri
    mmul = np.ascontiguousarray(mmul.astype(ml_dtypes.bfloat16))

    xTs = []
    xsums = []
    for b in range(B):
        xt = np.ascontiguousarray(x[b].T)                # [D, S]
        xTs.append(np.ascontiguousarray(xt.astype(ml_dtypes.bfloat16)))
        xs = xt.reshape(NCH, 128, 8, 256).sum(axis=3)    # [c, p, blk]
        xsums.append(np.ascontiguousarray(
            xs.transpose(1, 0, 2).astype(ml_dtypes.bfloat16)))
    per_p = {}
    for p in range(2):
        cs = slice(512 * p, 512 * p + 512)
        bq = bqkv[0:D][cs]
        bk = bqkv[D:2 * D][cs]
        bv = bqkv[2 * D:][cs]
        wo_p = np.ascontiguousarray(Wo[cs, :])
        bqk = np.zeros((128, 8), np.float32)
        for pr in range(NP):
            bqk[:, pr] = bq[128 * pr:128 * (pr + 1)]
            bqk[:, 4 + pr] = bk[128 * pr:128 * (pr + 1)]
        boc = bv @ wo_p + (bo if p == 0 else 0.0)
        bocol = np.ascontiguousarray(boc.reshape(8, 128).T)
        per_p[p] = {
            "wq": np.ascontiguousarray(Wqkv[:, cs].astype(ml_dtypes.bfloat16)),
            "wk": np.ascontiguousarray(
                Wqkv[:, D + 512 * p:D + 512 * p + 512].astype(ml_dtypes.bfloat16)),
            "wv": np.ascontiguousarray(
                Wqkv[:, 2 * D + 512 * p:2 * D + 512 * p + 512].astype(
                    ml_dtypes.bfloat16)),
            "wo": wo_p,
            "bqk": bqk,
            "bocol": bocol,
        }

    in_maps = []
    for core in range(8):
        b, p = core // 2, core % 2
        m = {"xT": xTs[b], "xsum": xsums[b], "mmul": mmul}
        m.update(per_p[p])
        in_maps.append(m)
    return in_maps


def assemble(results):
    out = np.zeros((B, S, D), np.float32)
    for b in range(B):
        out[b] = (results[2 * b]["outT"] + results[2 * b + 1]["outT"]).T
    return out


_CACHED = {}


def get_program():
    if "nc" not in _CACHED:
        _CACHED["nc"] = build_program()
    return _CACHED["nc"]


def kernel(x, Wqkv, bqkv, Wo, bo):
    from concourse.bass_utils import run_bass_kernel_spmd

    nc = get_program()
    in_maps = host_in_maps(x, Wqkv, bqkv, Wo, bo)
    res = run_bass_kernel_spmd(nc, in_maps, core_ids=list(range(8)))
    return assemble(res.results)
